# revision 8
# baseline (speedup 1.0000x reference)
"""Trainium2 Bass kernel for nn_LLMEmbeddingVQ.

Pipeline per core (4096 tokens):
  - x^T via PE transpose; fp16 hi/lo split of x and w_in.
  - z^T (fp16, scan-grade) and z rows (fp32-grade via 3-term fp16-pair matmul).
  - Scan: s[t,k] = z.c - cc/2 + (b.c) + 512 in PSUM via fp16 matmuls with a
    constant extra contraction row; DVE max8/max_index per 1024-quarter.
  - Global top-4 candidates per token; exact fp32 rescore via gathered
    codebook rows (cbx = [cb | cc]) and gpsimd dot products.
  - Outputs: indices, quantized = cb[idx] (gather), out = CW[idx] + b_out
    (gather from precomputed CW = cb @ w_out^T + b_out), aux partials.

kernel(**inputs) accepts the FULL unsharded inputs and returns
(out, indices, quantized, aux_loss) matching reference.py.
"""

import os
import sys

for _p in ("/opt/trn_rl_repo",):
    if _p not in sys.path and os.path.isdir(_p):
        sys.path.insert(0, _p)

from contextlib import ExitStack

import numpy as np

import concourse.bacc as bacc
import concourse.bass as bass
import concourse.mybir as mybir
import concourse.tile as tile
from concourse.bass import IndirectOffsetOnAxis
from concourse.bass_utils import run_bass_kernel_spmd

DT = mybir.dt
AF = mybir.ActivationFunctionType
ALU = mybir.AluOpType

B, C, N = 4, 32, 256
D_IN, D_EMB, K = 512, 1024, 4096
NCORES = 8
T_TOTAL = B * C * N
T_CORE = T_TOTAL // NCORES
CBX_W = 1028  # cb row (1024) + cc (1) + pad (3)
NEG_INF = -3.0e38


def build_nc(t_core=T_CORE, dots_on_gpsimd=False, stage=4):
    TG = 512
    NG = t_core // TG
    NT = TG // 128

    nc = bacc.Bacc("TRN2", target_bir_lowering=False, debug=False)
    f32, f16, u32, i32 = DT.float32, DT.float16, DT.uint32, DT.int32

    xs_d = nc.dram_tensor("xs", [t_core, D_IN], f32, kind="ExternalInput")
    cb_d = nc.dram_tensor("cb", [K, D_EMB], f32, kind="ExternalInput")
    cbt_d = nc.dram_tensor("cbt", [D_EMB, K], f16, kind="ExternalInput")
    wht_d = nc.dram_tensor("wht", [D_IN, D_EMB], f16, kind="ExternalInput")
    wlt_d = nc.dram_tensor("wlt", [D_IN, D_EMB], f16, kind="ExternalInput")
    bin_d = nc.dram_tensor("b_in", [D_EMB], f32, kind="ExternalInput")
    wot_d = nc.dram_tensor("wot", [D_EMB, D_IN], f16, kind="ExternalInput")
    bout_d = nc.dram_tensor("b_out", [D_IN], f32, kind="ExternalInput")

    out_d = nc.dram_tensor("out_s", [t_core, D_IN], f32, kind="ExternalOutput")
    idx_d = nc.dram_tensor("idx_s", [t_core, 1], i32, kind="ExternalOutput")
    qnt_d = nc.dram_tensor("qnt_s", [t_core, D_EMB], f32, kind="ExternalOutput")
    sts_d = nc.dram_tensor("sts_s", [128, NG * NT], f32, kind="ExternalOutput")

    with tile.TileContext(nc) as tc, ExitStack() as ctx:
        wp = ctx.enter_context(tc.tile_pool(name="wts", bufs=1))
        dramp = ctx.enter_context(tc.tile_pool(name="dram", bufs=1, space="DRAM"))

        cbx_t = dramp.tile([K, CBX_W], f32)
        cw_t = dramp.tile([K, D_IN], f32)

        # ---------- constants ----------
        ident = wp.tile([128, 128], f32)
        io_c = wp.tile([128, 128], i32)
        io_p = wp.tile([128, 128], i32)
        nc.gpsimd.iota(io_c[:], [[1, 128]], channel_multiplier=0)
        nc.gpsimd.iota(io_p[:], [[0, 128]], channel_multiplier=1)
        nc.vector.tensor_tensor(ident[:], io_c[:], io_p[:], op=ALU.is_equal)

        ones16 = wp.tile([1, 128], f16)
        nc.gpsimd.memset(ones16[:], 1.0)

        io32 = wp.tile([128, 32], i32)
        nc.gpsimd.iota(io32[:], [[1, 32]], channel_multiplier=0)
        io32f = wp.tile([128, 32], f32)
        nc.vector.tensor_copy(io32f[:], io32[:])
        qb32 = wp.tile([128, 32], i32)
        nc.gpsimd.iota(qb32[:], [[1024, 4], [0, 8]], channel_multiplier=0)
        qb32f = wp.tile([128, 32], f32)
        nc.vector.tensor_copy(qb32f[:], qb32[:])
        io4f = wp.tile([128, 4], f32)
        nc.vector.tensor_copy(io4f[:], io32[:, 0:4])

        # ---------- weights ----------
        wh_t = wp.tile([128, 4, D_EMB], f16)
        wl_t = wp.tile([128, 4, D_EMB], f16)
        nc.sync.dma_start(wh_t[:], wht_d.ap().rearrange("(dc p) e -> p dc e", p=128))
        nc.sync.dma_start(wl_t[:], wlt_d.ap().rearrange("(dc p) e -> p dc e", p=128))
        wo_t = wp.tile([128, 8, D_IN], f16)
        nc.sync.dma_start(wo_t[:], wot_d.ap().rearrange("(ec p) d -> p ec d", p=128))
        binr = wp.tile([128, D_EMB], f32)
        nc.sync.dma_start(binr[:], bass.AP(bin_d, 0, [[0, 128], [1, D_EMB]]))
        bin16 = wp.tile([128, 8], f16)
        nc.gpsimd.dma_start(bin16[:], bin_d.ap().rearrange("(ec p) -> p ec", p=128))
        bo32 = wp.tile([1, D_IN], f32)
        nc.sync.dma_start(bo32[:], bass.AP(bout_d, 0, [[0, 1], [1, D_IN]]))
        bo16 = wp.tile([1, D_IN], f16)
        nc.scalar.copy(bo16[:], bo32[:])

        ct_t = wp.tile([128, 8, K], f16)
        nc.sync.dma_start(ct_t[:], cbt_d.ap().rearrange("(ec p) k -> p ec k", p=128))

        # ---------- cc build + cbx fill ----------
        cc_sb = wp.tile([128, 32], f32)
        with tc.tile_pool(name="ccb", bufs=3) as ccp, \
             tc.tile_pool(name="ccps", bufs=2, space="PSUM") as ccps:
            for kc in range(32):
                cbrow = ccp.tile([128, D_EMB], f32, tag="cbrow")
                nc.sync.dma_start(cbrow[:], cb_d.ap()[kc * 128:(kc + 1) * 128, :])
                nc.sync.dma_start(cbx_t[:][kc * 128:(kc + 1) * 128, 0:D_EMB], cbrow[:])
                sq = ccp.tile([128, 8, 128], f32, tag="sq")
                nc.scalar.activation(
                    sq[:], cbrow[:].rearrange("p (a b) -> p a b", a=8), AF.Square)
                p8 = ccp.tile([128, 8], f32, tag="p8")
                nc.vector.tensor_reduce(p8[:], sq[:], axis=mybir.AxisListType.X, op=ALU.add)
                nc.vector.tensor_reduce(
                    cc_sb[:, kc:kc + 1], p8[:], axis=mybir.AxisListType.X, op=ALU.add)
            nc.sync.dma_start(
                cbx_t[:].rearrange("(kc p) e -> p kc e", p=128)[:, :, D_EMB:D_EMB + 1],
                cc_sb[:])

            # transpose cc to row layout: t32[kc, p] = cc_sb[p, kc]
            t32ps = ccps.tile([32, 128], f32)
            nc.tensor.transpose(t32ps[:], cc_sb[:], ident[:])
            t32 = ccp.tile([32, 128], f32, tag="t32")
            nc.scalar.copy(t32[:], t32ps[:])

            # b.c rows: bc[1, k] = sum_e b_in[e] * ct[e, k]
            negcc = []
            for r in range(8):
                bcps = ccps.tile([1, 512], f32)
                for ec in range(8):
                    nc.tensor.matmul(
                        bcps[:], bin16[:, ec:ec + 1], ct_t[:, ec, r * 512:(r + 1) * 512],
                        start=(ec == 0), stop=(ec == 7))
                bc32 = ccp.tile([1, 512], f32, tag="bc32")
                nc.scalar.copy(bc32[:], bcps[:])
                nr32 = ccp.tile([1, 512], f32, tag="nr32")
                for i in range(4):
                    nc.sync.dma_start(
                        nr32[0:1, i * 128:(i + 1) * 128], t32[4 * r + i:4 * r + i + 1, :])
                tmp = ccp.tile([1, 512], f32, tag="tmpnc")
                nc.vector.scalar_tensor_tensor(
                    tmp[:], nr32[:], -0.5, bc32[:], op0=ALU.mult, op1=ALU.add)
                nrow = wp.tile([1, 512], f16, tag=f"negcc{r}")
                nc.vector.tensor_scalar(
                    tmp[:], tmp[:], 512.0, None, op0=ALU.add)
                nc.vector.tensor_copy(nrow[:], tmp[:])
                negcc.append(nrow)

        # ---------- CW build ----------
        with tc.tile_pool(name="cwb", bufs=3) as cwp, \
             tc.tile_pool(name="cwps", bufs=2, space="PSUM") as cwps:
            for kc in range(32):
                ps = cwps.tile([128, D_IN], f32)
                for ec in range(8):
                    nc.tensor.matmul(
                        ps[:], ct_t[:, ec, kc * 128:(kc + 1) * 128], wo_t[:, ec, :],
                        start=(ec == 0), stop=False)
                nc.tensor.matmul(ps[:], ones16[:], bo16[:], start=False, stop=True)
                cwsb = cwp.tile([128, D_IN], f32)
                nc.scalar.copy(cwsb[:], ps[:])
                nc.sync.dma_start(cw_t[:][kc * 128:(kc + 1) * 128, :], cwsb[:])

        # ---------- main pools ----------
        xrp = ctx.enter_context(tc.tile_pool(name="xrow", bufs=2))
        xtp = ctx.enter_context(tc.tile_pool(name="xt", bufs=1))
        xhp = ctx.enter_context(tc.tile_pool(name="xh", bufs=2))
        ztp = ctx.enter_context(tc.tile_pool(name="zt", bufs=2))
        zrp = ctx.enter_context(tc.tile_pool(name="zr", bufs=2))
        smp = ctx.enter_context(tc.tile_pool(name="smalls", bufs=2))
        cgp = ctx.enter_context(tc.tile_pool(name="cand", bufs=5))
        dmp = ctx.enter_context(tc.tile_pool(name="dump", bufs=1))
        qgp = ctx.enter_context(tc.tile_pool(name="qrow", bufs=2))
        ogp = ctx.enter_context(tc.tile_pool(name="orow", bufs=2))

        xps = ctx.enter_context(tc.tile_pool(name="xps", bufs=1, space="PSUM"))
        zpsp = ctx.enter_context(tc.tile_pool(name="zps", bufs=2, space="PSUM"))
        zrpsp = ctx.enter_context(tc.tile_pool(name="zrps", bufs=1, space="PSUM"))
        scp = ctx.enter_context(tc.tile_pool(name="scps", bufs=2, space="PSUM"))

        stats = wp.tile([128, NG * NT], f32)
        nc.vector.memset(stats[:], 0.0)

        for g in range(NG):
            tok0 = g * TG
            # ---- x^T + fp16 split
            xT32 = xtp.tile([128, 4, TG], f32)
            for ti in range(NT):
                xrow = xrp.tile([128, D_IN], f32)
                nc.sync.dma_start(
                    xrow[:], xs_d.ap()[tok0 + ti * 128: tok0 + (ti + 1) * 128, :])
                tps = xps.tile([128, 512], f32)
                for dc in range(4):
                    nc.tensor.transpose(
                        tps[:, dc * 128:(dc + 1) * 128],
                        xrow[:, dc * 128:(dc + 1) * 128], ident[:])
                nc.scalar.copy(
                    xT32[:, :, ti * 128:(ti + 1) * 128],
                    tps[:].rearrange("p (dc t) -> p dc t", dc=4))
            xhT = xhp.tile([128, 4, TG], f16, tag="xhT")
            xlT = xhp.tile([128, 4, TG], f16, tag="xlT")
            for dc in range(4):
                nc.scalar.copy(xhT[:, dc, :], xT32[:, dc, :])
                nc.vector.tensor_tensor(
                    xlT[:, dc, :], xT32[:, dc, :], xhT[:, dc, :], op=ALU.subtract)

            # ---- z^T (scan grade, fp16, no bias)
            zt16 = ztp.tile([128, 8, TG], f16)
            for ec in range(8):
                zps = zpsp.tile([128, TG], f32)
                for dc in range(4):
                    nc.tensor.matmul(
                        zps[:], wh_t[:, dc, ec * 128:(ec + 1) * 128], xhT[:, dc, :],
                        start=(dc == 0), stop=(dc == 3))
                nc.scalar.copy(zt16[:, ec, :], zps[:])

            for ti in range(NT):
                tok = tok0 + ti * 128
                # ---- z rows (fp32 grade)
                zr = zrp.tile([128, D_EMB], f32)
                for eh in range(2):
                    zrps = zrpsp.tile([128, 512], f32)
                    nmm = 12
                    i = 0
                    for dc in range(4):
                        for (xa, wa) in ((xhT, wh_t), (xhT, wl_t), (xlT, wh_t)):
                            nc.tensor.matmul(
                                zrps[:],
                                xa[:, dc, ti * 128:(ti + 1) * 128],
                                wa[:, dc, eh * 512:(eh + 1) * 512],
                                start=(i == 0), stop=(i == nmm - 1))
                            i += 1
                    nc.vector.tensor_tensor(
                        zr[:, eh * 512:(eh + 1) * 512], zrps[:],
                        binr[:, eh * 512:(eh + 1) * 512], op=ALU.add)

                # ---- scan
                m8s = smp.tile([128, 4, 8], f32, tag="m8s")
                i8s = smp.tile([128, 4, 8], u32, tag="i8s")
                for kq in range(4):
                    scps = scp.tile([128, 1024], f32)
                    for h in range(2):
                        col0 = kq * 1024 + h * 512
                        for ec in range(8):
                            nc.tensor.matmul(
                                scps[:, h * 512:(h + 1) * 512],
                                zt16[:, ec, ti * 128:(ti + 1) * 128],
                                ct_t[:, ec, col0:col0 + 512],
                                start=(ec == 0), stop=False)
                        nc.tensor.matmul(
                            scps[:, h * 512:(h + 1) * 512],
                            ones16[:], negcc[kq * 2 + h][:], start=False, stop=True)
                    nc.vector.max(m8s[:, kq, :], scps[:])
                    nc.vector.max_index(i8s[:, kq, :], m8s[:, kq, :], scps[:])

                if stage < 2:
                    nc.sync.dma_start(
                        qnt_d.ap()[tok:tok + 128, 0:8],
                        m8s[:].rearrange("p a b -> p (a b)")[:, 0:8])
                    continue
                # ---- merge to global top-4
                i32f = smp.tile([128, 32], f32, tag="i32f")
                nc.vector.tensor_copy(i32f[:], i8s[:].rearrange("p a b -> p (a b)"))
                kgf = smp.tile([128, 32], f32, tag="kgf")
                nc.vector.tensor_tensor(kgf[:], i32f[:], qb32f[:], op=ALU.add)
                g8 = smp.tile([128, 8], f32, tag="g8")
                pos8 = smp.tile([128, 8], u32, tag="pos8")
                m32 = m8s[:].rearrange("p a b -> p (a b)")
                nc.vector.max(g8[:], m32)
                nc.vector.max_index(pos8[:], g8[:], m32)
                pos8f = smp.tile([128, 8], f32, tag="pos8f")
                nc.vector.tensor_copy(pos8f[:], pos8[:])
                idx4f = smp.tile([128, 4], f32, tag="idx4f")
                oh = smp.tile([128, 32], f32, tag="oh")
                ohk = smp.tile([128, 32], f32, tag="ohk")
                for j in range(4):
                    nc.vector.tensor_scalar(
                        oh[:], io32f[:], pos8f[:, j:j + 1], None, op0=ALU.is_equal)
                    nc.vector.tensor_tensor(ohk[:], oh[:], kgf[:], op=ALU.mult)
                    nc.vector.tensor_reduce(
                        idx4f[:, j:j + 1], ohk[:], axis=mybir.AxisListType.X, op=ALU.add)
                idx4u = smp.tile([128, 4], u32, tag="idx4u")
                nc.vector.tensor_copy(idx4u[:], idx4f[:])

                if stage < 3:
                    nc.sync.dma_start(qnt_d.ap()[tok:tok + 128, 0:4], idx4f[:])
                    continue
                # ---- gather candidates + exact rescore
                cands = [cgp.tile([128, CBX_W], f32, tag="cand", name=f"cand{j}")
                         for j in range(4)]
                for j in range(4):
                    nc.gpsimd.indirect_dma_start(
                        cands[j][:], None, cbx_t[:],
                        IndirectOffsetOnAxis(ap=idx4u[:, j:j + 1], axis=0))
                f4 = smp.tile([128, 4], f32, tag="f4")
                eng = nc.gpsimd if dots_on_gpsimd else nc.vector
                for j in range(4):
                    dump = dmp.tile([128, D_EMB], f32, tag="dump")
                    sj = smp.tile([128, 1], f32, tag="sj")
                    eng.scalar_tensor_tensor(
                        dump[:], zr[:], 1.0, cands[j][:, 0:D_EMB],
                        op0=ALU.mult, op1=ALU.mult, accum_out=sj[:])
                    nc.vector.scalar_tensor_tensor(
                        f4[:, j:j + 1], sj[:], -2.0, cands[j][:, D_EMB:D_EMB + 1],
                        op0=ALU.mult, op1=ALU.add)

                # ---- pick winner
                F8 = smp.tile([128, 8], f32, tag="F8")
                nc.vector.memset(F8[:], NEG_INF)
                nc.vector.tensor_scalar(F8[:, 0:4], f4[:], -1.0, None, op0=ALU.mult)
                gf = smp.tile([128, 8], f32, tag="gf")
                posf = smp.tile([128, 8], u32, tag="posf")
                nc.vector.max(gf[:], F8[:])
                nc.vector.max_index(posf[:], gf[:], F8[:])
                posff = smp.tile([128, 1], f32, tag="posff")
                nc.vector.tensor_copy(posff[:], posf[:, 0:1])
                oh4 = smp.tile([128, 4], f32, tag="oh4")
                nc.vector.tensor_scalar(
                    oh4[:], io4f[:], posff[:], None, op0=ALU.is_equal)
                ohi = smp.tile([128, 4], f32, tag="ohi")
                nc.vector.tensor_tensor(ohi[:], oh4[:], idx4f[:], op=ALU.mult)
                idxsf = smp.tile([128, 1], f32, tag="idxsf")
                nc.vector.tensor_reduce(
                    idxsf[:], ohi[:], axis=mybir.AxisListType.X, op=ALU.add)
                idxi = smp.tile([128, 1], i32, tag="idxi")
                nc.vector.tensor_copy(idxi[:], idxsf[:])
                idxu = smp.tile([128, 1], u32, tag="idxu")
                nc.vector.tensor_copy(idxu[:], idxsf[:])
                nc.sync.dma_start(idx_d.ap()[tok:tok + 128, :], idxi[:])

                # ---- aux: d* = zz - gf[0]
                dumpa = dmp.tile([128, D_EMB], f32, tag="dumpa")
                zz = smp.tile([128, 1], f32, tag="zz")
                nc.scalar.activation(dumpa[:], zr[:], AF.Square, accum_out=zz[:])
                nc.vector.scalar_tensor_tensor(
                    stats[:, g * NT + ti: g * NT + ti + 1], gf[:, 0:1], -1.0, zz[:],
                    op0=ALU.mult, op1=ALU.add)

                if stage < 4:
                    nc.sync.dma_start(qnt_d.ap()[tok:tok + 128, 0:4], f4[:])
                    nc.sync.dma_start(qnt_d.ap()[tok:tok + 128, 4:5], idxsf[:])
                    continue
                # ---- output gathers
                qrow = qgp.tile([128, D_EMB], f32)
                nc.gpsimd.indirect_dma_start(
                    qrow[:], None, cbx_t[:],
                    IndirectOffsetOnAxis(ap=idxu[:], axis=0))
                nc.sync.dma_start(qnt_d.ap()[tok:tok + 128, :], qrow[:])
                orow = ogp.tile([128, D_IN], f32)
                nc.gpsimd.indirect_dma_start(
                    orow[:], None, cw_t[:],
                    IndirectOffsetOnAxis(ap=idxu[:], axis=0))
                nc.sync.dma_start(out_d.ap()[tok:tok + 128, :], orow[:])

        nc.sync.dma_start(sts_d.ap(), stats[:])

    nc.compile()
    return nc


_NC_CACHE = {}


def _get_nc(t_core=T_CORE):
    key = t_core
    if key not in _NC_CACHE:
        _NC_CACHE[key] = build_nc(t_core)
    return _NC_CACHE[key]


def prep_inputs(x, w_in, b_in, w_out, b_out, codebook, t_core=T_CORE, ncores=NCORES):
    x = np.asarray(x, dtype=np.float32)
    w_in = np.asarray(w_in, dtype=np.float32)
    b_in = np.asarray(b_in, dtype=np.float32)
    w_out = np.asarray(w_out, dtype=np.float32)
    b_out = np.asarray(b_out, dtype=np.float32)
    cb = np.ascontiguousarray(np.asarray(codebook, dtype=np.float32))

    cbt = np.ascontiguousarray(cb.T).astype(np.float16)
    wt = np.ascontiguousarray(w_in.T)  # [D_IN, D_EMB]
    wht = wt.astype(np.float16)
    wlt = (wt - wht.astype(np.float32)).astype(np.float16)
    wot = np.ascontiguousarray(w_out.T).astype(np.float16)  # [D_EMB, D_IN]

    xf = np.ascontiguousarray(x.reshape(-1, D_IN))
    in_maps = []
    for c in range(ncores):
        in_maps.append({
            "xs": np.ascontiguousarray(xf[c * t_core:(c + 1) * t_core]),
            "cb": cb,
            "cbt": cbt,
            "wht": wht,
            "wlt": wlt,
            "b_in": b_in,
            "wot": wot,
            "b_out": b_out,
        })
    return in_maps


def kernel(x, w_in, b_in, w_out, b_out, codebook):
    nc = _get_nc()
    in_maps = prep_inputs(x, w_in, b_in, w_out, b_out, codebook)
    res = run_bass_kernel_spmd(nc, in_maps, core_ids=list(range(NCORES)))
    outs = [r["out_s"] for r in res.results]
    idxs = [r["idx_s"] for r in res.results]
    qnts = [r["qnt_s"] for r in res.results]
    stss = [r["sts_s"] for r in res.results]

    out = np.concatenate(outs, axis=0).reshape(B, C, N, D_IN)
    indices = np.concatenate(idxs, axis=0).reshape(B, C * N).astype(np.int32)
    quantized = np.concatenate(qnts, axis=0).reshape(B, C, N, D_EMB)
    total = np.sum([s.astype(np.float64).sum() for s in stss])
    aux = np.float32(total / (T_TOTAL * D_EMB))
    return out, indices, quantized, aux
